# revision 57
# baseline (speedup 1.0000x reference)
"""Trainium2 Bass kernel for a minimal transformer block (B=2, T=2048, C=1024,
H=16, Dh=64, F=4096), sharded over 8 NeuronCores.

Sharding: data-parallel over batch (2 groups of 4 cores) x sequence-parallel
over tokens within each batch (512 tokens per core). Each core computes
Q/K/V only for its own 512 tokens; K and V are exchanged within the 4-core
group via two HBM AllGathers (K first so the score matmuls can start while
V is still in flight). LayerNorm mean-centering is applied to x in place
(one PE broadcast + 8 vector adds), so the projection chains carry no rank-1
mean-correction matmuls. Softmax denominators are collected across all 16
heads and inverted in one batched reciprocal.

Everything on-chip is feature-major ([features, tokens]); the host transposes
inputs/outputs and pre-transposes/casts weights to bf16.
"""

import sys

if "/opt/trn_rl_repo" not in sys.path:
    sys.path.insert(0, "/opt/trn_rl_repo")

import numpy as np

D_MODEL = 1024
N_HEAD = 16
HEAD_DIM = 64
D_FF = 4096
B = 2
T = 2048
N_CORES = 8
GROUPS = 4          # cores per batch
TQ = T // GROUPS    # own tokens per core = 512
P = 128
NCC = D_MODEL // P  # 8 C-chunks
NKC = T // P        # 16 k-chunks of 128
NOC = TQ // P       # 4 own k-chunks
NFC = D_FF // P     # 32 f-chunks of 128

# bias-table column layout ([128, 64] f32)
QB, OB, B1, B2 = 0, 8, 16, 48

RG = [[0, 1, 2, 3], [4, 5, 6, 7]]

_cache = {}


def _build():
    import concourse.bass as bass
    import concourse.tile as tile
    from concourse import bacc, mybir

    f32 = mybir.dt.float32
    bf16 = mybir.dt.bfloat16
    f8 = mybir.dt.float8e4
    AF = mybir.ActivationFunctionType
    OP = mybir.AluOpType
    DR = mybir.MatmulPerfMode.DoubleRow

    nc = bacc.Bacc("TRN2", target_bir_lowering=False, debug=False,
                   num_devices=N_CORES)

    x_d = nc.dram_tensor("x_fm", [D_MODEL, TQ], bf16,
                         kind="ExternalInput").ap()
    xo_d = nc.dram_tensor("x_own", [D_MODEL, TQ], f32,
                          kind="ExternalInput").ap()
    qkvw_d = nc.dram_tensor("qkv_wT", [D_MODEL, 3 * D_MODEL], bf16,
                            kind="ExternalInput").ap()
    ow_d = nc.dram_tensor("o_wT", [D_MODEL, D_MODEL], bf16,
                          kind="ExternalInput").ap()
    w1_d = nc.dram_tensor("w1T", [D_MODEL, D_FF], bf16,
                          kind="ExternalInput").ap()
    w2_d = nc.dram_tensor("w2T", [D_FF, D_MODEL], bf16,
                          kind="ExternalInput").ap()
    bias_d = nc.dram_tensor("biases", [P, 64], f32, kind="ExternalInput").ap()
    sel_d = nc.dram_tensor("sel16", [16, NCC * P], bf16,
                           kind="ExternalInput").ap()
    out_d = nc.dram_tensor("out", [D_MODEL, TQ], f32,
                           kind="ExternalOutput").ap()

    with tile.TileContext(nc) as tc:
        _body(tc, bass, mybir, f32, bf16, f8, AF, OP, DR, x_d, xo_d, qkvw_d,
              ow_d, w1_d, w2_d, bias_d, sel_d, out_d)

    nc.compile()
    return nc


def _body(tc, bass, mybir, f32, bf16, f8, AF, OP, DR, x_d, xo_d, qkvw_d,
          ow_d, w1_d, w2_d, bias_d, sel_d, out_d):
    nc = tc.nc
    from contextlib import ExitStack

    ctx = ExitStack()
    with ctx:
        # ---- persistent arenas (stack-allocated; open the whole kernel)
        const_pool = ctx.enter_context(tc.tile_pool(name="const", bufs=1))
        x_pool = ctx.enter_context(tc.tile_pool(name="xp", bufs=1))
        x2_pool = ctx.enter_context(tc.tile_pool(name="x2", bufs=1))
        karena = ctx.enter_context(tc.tile_pool(name="karena", bufs=1))
        v_pool = ctx.enter_context(tc.tile_pool(name="v", bufs=1))
        kown_pool = ctx.enter_context(tc.tile_pool(name="kown", bufs=1))
        vown_pool = ctx.enter_context(tc.tile_pool(name="vown", bufs=1))
        q_pool = ctx.enter_context(tc.tile_pool(name="q", bufs=1))
        wa_pool = ctx.enter_context(tc.tile_pool(name="wa", bufs=1))
        wb_pool = ctx.enter_context(tc.tile_pool(name="wb", bufs=1))
        sln_pool = ctx.enter_context(tc.tile_pool(name="sln", bufs=1))
        xc_pool = ctx.enter_context(tc.tile_pool(name="xc", bufs=1))
        h2_pool = ctx.enter_context(tc.tile_pool(name="h2", bufs=1))
        # one DRAM pool per collective tensor: coarse DRAM dependency
        # tracking otherwise serializes the V input DMAs behind the K gather
        dram_ki = ctx.enter_context(tc.tile_pool(name="dki", bufs=1,
                                                 space="DRAM"))
        dram_ko = ctx.enter_context(tc.tile_pool(name="dko", bufs=1,
                                                 space="DRAM"))
        dram_vi = ctx.enter_context(tc.tile_pool(name="dvi", bufs=1,
                                                 space="DRAM"))
        dram_vo = ctx.enter_context(tc.tile_pool(name="dvo", bufs=1,
                                                 space="DRAM"))

        # K/Q/V/es are fp8e4: halves both gather payloads, and V/es pack
        # adjacent key chunks (2j, 2j+1) into the free dim so the PV matmuls
        # contract 2 key rows per partition at fp8 DoubleRow rate.
        cc_k_in = dram_ki.tile([D_MODEL, TQ], f8, tag="cki", name="cc_k_in")
        cc_k_out = dram_ko.tile([GROUPS * D_MODEL, TQ], f8, tag="cko",
                                name="cc_k_out")
        cc_v_in = dram_vi.tile([2 * P, 2 * N_HEAD * 65], f8, tag="cvi",
                               name="cc_v_in")
        cc_v_out = dram_vo.tile([8 * P, 2 * N_HEAD * 65], f8, tag="cvo",
                                name="cc_v_out")

        # raw x (bf16, feature-major, own tokens); first in the DMA queue
        # since it gates the LN1 stats
        xb = []
        for ci in range(NCC):
            xt = x_pool.tile([P, TQ], bf16, tag=f"x{ci}", name=f"x{ci}")
            nc.sync.dma_start(xt[:], x_d[ci * P:(ci + 1) * P, :])
            xb.append(xt)

        bias_sb = const_pool.tile([P, 64], f32, tag="bias", name="bias")
        nc.sync.dma_start(bias_sb[:], bias_d[:])
        ones_bf = const_pool.tile([P, 1], bf16, tag="ones_bf", name="ones_bf")
        nc.vector.memset(ones_bf[:], 1.0)
        ones_row = const_pool.tile([1, P], bf16, tag="ones_row",
                                   name="ones_row")
        nc.vector.memset(ones_row[:], 1.0)
        # sel16 column block ct routes recip rows (2ct, 2ct+1) to the lower/
        # upper 64 output partitions: sel16[i, ct*128+m] = 1 iff i == 2*ct +
        # (m >= 64). Stationary operand of the denominator broadcast matmuls.
        sel16 = const_pool.tile([16, NCC * P], bf16, tag="sel16",
                                name="sel16")
        nc.sync.dma_start(sel16[:], sel_d[:])

        def bcol(base, i):
            return bias_sb[:, base + i:base + i + 1]

        # K weights first (K projection gates the gather), then V weights
        wk = []
        for ci in range(NCC):
            wt = wb_pool.tile([P, D_MODEL], bf16, tag=f"wb{ci}",
                              name=f"wk{ci}")
            nc.sync.dma_start(wt[:], qkvw_d[ci * P:(ci + 1) * P,
                                            D_MODEL:2 * D_MODEL])
            wk.append(wt)
        wv = []
        for ci in range(NCC):
            wt = wa_pool.tile([P, D_MODEL], bf16, tag=f"wa{ci}",
                              name=f"wv{ci}")
            nc.sync.dma_start(wt[:], qkvw_d[ci * P:(ci + 1) * P,
                                            2 * D_MODEL:3 * D_MODEL])
            wv.append(wt)

        # LN1 per-token stats over own tokens: s = rsqrt(var+eps) row and
        # nmu = -mu; x is then centered in place so every projection chain
        # is a plain 8-step matmul chain.
        nmu_row = sln_pool.tile([1, TQ], bf16, tag="nmu", name="nmu_row")
        s_row = sln_pool.tile([1, TQ], bf16, tag="srow", name="s_row")
        s_b = sln_pool.tile([P, TQ], bf16, tag="sb", name="s_b")
        s_col = sln_pool.tile([P, NOC], f32, tag="scol", name="s_col")
        nmu2_row = sln_pool.tile([1, TQ], bf16, tag="nmu2", name="nmu2_row")
        s2_bb = sln_pool.tile([P, TQ], bf16, tag="s2bb", name="s2_bb")

        with tc.tile_pool(name="xsq", bufs=4) as xsq_pool, \
             tc.tile_pool(name="ln1ps", bufs=1, space="PSUM") as lnps, \
             tc.tile_pool(name="ln1bc", bufs=2, space="PSUM") as lnbc, \
             tc.tile_pool(name="ln1t", bufs=1) as lnt:

            st = lnps.tile([33, TQ], f32, tag="st", name="st")
            for ci in range(NCC):
                xsq = xsq_pool.tile([P, TQ], bf16, tag="xsq", name="xsq")
                nc.vector.tensor_mul(xsq[:], xb[ci][:], xb[ci][:])
                nc.tensor.matmul(st[0:1, :], ones_bf[:], xb[ci][:],
                                 start=(ci == 0), stop=(ci == NCC - 1))
                nc.tensor.matmul(st[32:33, :], ones_bf[:], xsq[:],
                                 start=(ci == 0), stop=(ci == NCC - 1))

            inv_n = 1.0 / D_MODEL
            mu = lnt.tile([1, TQ], f32, tag="mu", name="mu")
            nc.vector.tensor_scalar_mul(mu[:], st[0:1, :], inv_n)
            mu2 = lnt.tile([1, TQ], f32, tag="mu2", name="mu2")
            nc.vector.tensor_mul(mu2[:], mu[:], mu[:])
            vpe = lnt.tile([1, TQ], f32, tag="vpe", name="vpe")
            nc.vector.tensor_scalar(vpe[:], st[32:33, :], inv_n, 1e-5,
                                    OP.mult, OP.add)
            nc.vector.tensor_sub(vpe[:], vpe[:], mu2[:])
            rec1 = lnt.tile([1, TQ], f32, tag="rec", name="rec1")
            nc.vector.reciprocal_approx_fast(rec1[:], vpe[:])
            with nc.allow_low_precision(reason="bf16 LN scale rows"):
                nc.scalar.sqrt(s_row[:], rec1[:])
                nc.vector.tensor_scalar_mul(nmu_row[:], mu[:], -1.0)

            # broadcasts: nmu and s across all 128 partitions (PE), and
            # token-major s for the V drain (4 tiny transposing matmuls)
            nmu_ps = lnbc.tile([P, TQ], f32, tag="bc", name="nmu_ps")
            nc.tensor.matmul(nmu_ps[:], ones_row[:], nmu_row[:])
            nmu_b = lnt.tile([P, TQ], bf16, tag="nmub", name="nmu_b")
            nc.scalar.copy(nmu_b[:], nmu_ps[:])
            s_ps = lnbc.tile([P, TQ], f32, tag="bc", name="s_ps")
            nc.tensor.matmul(s_ps[:], ones_row[:], s_row[:])
            nc.scalar.copy(s_b[:], s_ps[:])
            scol_ps = lnbc.tile([P, NOC], f32, tag="sc2", name="scol_ps")
            for tk in range(NOC):
                nc.tensor.matmul(scol_ps[:, tk:tk + 1],
                                 s_row[:, tk * P:(tk + 1) * P],
                                 ones_row[:, 0:1])
            nc.vector.tensor_copy(s_col[:], scol_ps[:])

            # center x in place: xc = x - mu (stats above already read xb)
            for ci in range(NCC):
                nc.vector.tensor_add(xb[ci][:], xb[ci][:], nmu_b[:])

        # ---------------- K/V own projections + gathers ----------------
        k8own = [kown_pool.tile([P, TQ], f8, tag=f"ko{i}", name=f"ko{i}")
                 for i in range(NCC)]
        v8own = [vown_pool.tile([P, 2 * N_HEAD * 65], f8, tag=f"vo{jl}",
                                name=f"vo{jl}") for jl in range(2)]
        v8o4 = [v.rearrange("p (i h s) -> p i h s", i=2, s=65)
                for v in v8own]

        # K/Q DoubleRow layout: tile t holds head 2t at partitions 0-31 and
        # head 2t+1 at partitions 64-95 (legal matmul bases), head-dim split
        # d = 32*i + p%32 with the pair index i in the free dim
        k8 = [karena.tile([P, 2 * T], f8, tag=f"k{i}", name=f"k8_{i}")
              for i in range(NCC)]
        k83 = [k.rearrange("p (i t) -> p i t", i=2) for k in k8]
        v8 = [v_pool.tile([P, 2 * N_HEAD * 65], f8, tag=f"v{j}",
                          name=f"v8_{j}") for j in range(8)]
        v84 = [v.rearrange("p (i h s) -> p i h s", i=2, s=65) for v in v8]

        with tc.tile_pool(name="qkvps", bufs=4, space="PSUM") as qkv_ps:
            # K own: k = s * (Wk'.xc)   (K bias cancels in softmax)
            with nc.allow_low_precision(reason="fp8 attention operands"):
                for co in range(NCC):
                    ps = qkv_ps.tile([P, TQ], f32, tag="ps", name="qkv_ps")
                    for ci in range(NCC):
                        nc.tensor.matmul(
                            ps[:], wk[ci][:, co * P:(co + 1) * P],
                            xb[ci][:], start=(ci == 0),
                            stop=(ci == NCC - 1))
                    nc.vector.tensor_mul(k8own[co][:], ps[:], s_b[:])
                    nc.gpsimd.dma_start(cc_k_in[co * P:(co + 1) * P, :],
                                        k8own[co][:])
            nc.gpsimd.collective_compute(
                "AllGather", mybir.AluOpType.bypass, replica_groups=RG,
                ins=[cc_k_in.opt()], outs=[cc_k_out.opt()])

            # Q weight DMAs emitted now: on the sync queue they only wait
            # for the K chains (WAR on the wb slots)
            wq = []
            for ci in range(NCC):
                wt = wb_pool.tile([P, D_MODEL], bf16, tag=f"wb{ci}",
                                  name=f"wq{ci}")
                nc.sync.dma_start(wt[:], qkvw_d[ci * P:(ci + 1) * P,
                                                0:D_MODEL])
                wq.append(wt)

            # V own, token-major with ones column: v = s_t * (xc.Wv);
            # chunk pairs (2j, 2j+1) interleave into the DoubleRow free dim
            with nc.allow_low_precision(reason="fp8 attention operands"):
                for jl in range(2):
                    nc.vector.memset(v8o4[jl][:, :, :, 64:65], 1.0)
                for c in range(NOC):
                    jl, il = c // 2, c % 2
                    tsl = slice(c * P, (c + 1) * P)
                    for vh in range(2):
                        ps = qkv_ps.tile([P, TQ], f32, tag="ps",
                                         name="qkv_ps")
                        for ci in range(NCC):
                            nc.tensor.matmul(
                                ps[:], xb[ci][:, tsl],
                                wv[ci][:, vh * 512:(vh + 1) * 512],
                                start=(ci == 0), stop=(ci == NCC - 1))
                        src = ps.rearrange("p (h d) -> p h d", d=64)
                        nc.vector.tensor_scalar_mul(
                            v8o4[jl][:, il, vh * 8:(vh + 1) * 8, 0:64],
                            src[:], s_col[:, c:c + 1])
                for jl in range(2):
                    nc.sync.dma_start(cc_v_in[jl * P:(jl + 1) * P, :],
                                      v8own[jl][:])
            nc.gpsimd.collective_compute(
                "AllGather", mybir.AluOpType.bypass, replica_groups=RG,
                ins=[cc_v_in.opt()], outs=[cc_v_out.opt()])

            # Q (own tokens): q = s * (Wq'.xc) + bq — staged through DRAM so
            # a DMA can remap dense projection partitions into the DoubleRow
            # layout (vector drains can't permute partitions); the k8own
            # slots are dead by now and serve as staging
            q_dram = dram_ki.tile([D_MODEL, TQ], f8, tag="qdr",
                                  name="q_dram")
            qt = sln_pool.tile([P, TQ], f32, tag="qt", name="qt")
            with nc.allow_low_precision(reason="fp8 attention operands"):
                for co in range(NCC):
                    ps = qkv_ps.tile([P, TQ], f32, tag="ps", name="qkv_ps")
                    for ci in range(NCC):
                        nc.tensor.matmul(
                            ps[:], wq[ci][:, co * P:(co + 1) * P],
                            xb[ci][:], start=(ci == 0),
                            stop=(ci == NCC - 1))
                    nc.vector.tensor_mul(qt[:], ps[:], s_b[:])
                    qst = kown_pool.tile([P, TQ], f8, tag=f"ko{co}",
                                         name=f"qst{co}")
                    nc.vector.tensor_scalar_add(qst[:], qt[:],
                                                bcol(QB, co))
                    nc.sync.dma_start(q_dram[co * P:(co + 1) * P, :],
                                      qst[:])
            q8 = [q_pool.tile([P, 2 * TQ], f8, tag=f"q{t}", name=f"q8_{t}")
                  for t in range(NCC)]
            q83 = [q.rearrange("p (i t) -> p i t", i=2) for q in q8]
            qv = q_dram.rearrange("(t hp dd i) tok -> t hp dd i tok",
                                  t=8, hp=2, dd=32, i=2)
            for t in range(NCC):
                for hp in range(2):
                    nc.sync.dma_start(
                        q83[t][64 * hp:64 * hp + 32, :, :], qv[t, hp])

            # prefetch the first W1 quarter into the dead Q-weight slots so
            # FFN1 doesn't wait on the K arena (busy until the last scores)
            w1q = []
            for ci in range(NCC):
                wt = wb_pool.tile([P, 1024], bf16, tag=f"wb{ci}",
                                  name=f"w1q{ci}")
                nc.sync.dma_start(wt[:], w1_d[ci * P:(ci + 1) * P, 0:1024])
                w1q.append(wt)

            # pull gathered K/V into SBUF (gpsimd queue: ordered after the
            # collectives; K first — scores need it before PV needs V).
            # K rows are (t, hp, dd, i)-ordered, so one strided DMA per
            # (tile, head-half) scatters all 4 rank blocks into place.
            ckv = cc_k_out.rearrange(
                "(r t hp dd i) tok -> t hp dd i r tok",
                r=4, t=8, hp=2, dd=32, i=2)
            for t in range(NCC):
                for hp in range(2):
                    dst = k8[t].rearrange("p (i r tok) -> p i r tok",
                                          i=2, r=4)
                    nc.gpsimd.dma_start(
                        dst[64 * hp:64 * hp + 32, :, :, :], ckv[t, hp])
            for j in range(8):
                nc.gpsimd.dma_start(v8[j][:],
                                    cc_v_out[j * P:(j + 1) * P, :])

        # ---------------- attention + output projection ----------------
        x2 = [x2_pool.tile([P, TQ], bf16, tag=f"x2{i}", name=f"x2_{i}")
              for i in range(NCC)]
        xbc = [xc_pool.tile([P, TQ], bf16, tag=f"xc{i}", name=f"xb2c{i}")
               for i in range(NCC)]

        with tc.tile_pool(name="attn", bufs=1) as attn_pool:
            attn_sb = [attn_pool.tile([P, TQ], bf16, tag=f"a{i}",
                                      name=f"attn{i}") for i in range(NCC)]
            # per-head softmax denominators: engine writes must start at
            # partition 0/32/64/96, so each head's row is staged at
            # partition 0 (in a dead x slot) and DMA'd onto partition hd;
            # one batched reciprocal then covers all 16 heads
            dden = attn_pool.tile([16, TQ], bf16, tag="dd", name="dden")

            # prefetch O weights during attention
            ow = []
            with tc.tile_pool(name="ow", bufs=1) as ow_pool:
                for ci in range(NCC):
                    wt = ow_pool.tile([P, D_MODEL], bf16, tag=f"ow{ci}",
                                      name=f"ow{ci}")
                    nc.sync.dma_start(wt[:], ow_d[ci * P:(ci + 1) * P, :])
                    ow.append(wt)

                with tc.tile_pool(name="es", bufs=32) as es_pool, \
                     tc.tile_pool(name="scps", bufs=3, space="PSUM") as sc_ps, \
                     tc.tile_pool(name="pvps", bufs=2, space="PSUM") as pv_psp:

                    # software-pipelined head loop (depth 3): the PE queue is
                    # in-order, so PV for head h must be emitted AFTER the
                    # scores of heads h+1..h+3 — otherwise a PV waiting on
                    # the V gather blocks later heads' score matmuls too
                    PIPE = 3
                    es_store = {}
                    for step in range(N_HEAD + PIPE):
                        if step < N_HEAD:
                            hd = step
                            t, bp = hd // 2, 64 * (hd % 2)
                            qsl = q83[t][bp:bp + 32, :, :]
                            es = []
                            for tp in range(NKC // 2):
                                ps = sc_ps.tile([P, 2 * TQ], f32, tag="sc",
                                                name="sc_ps")
                                for half in range(2):
                                    c = 2 * tp + half
                                    nc.tensor.matmul(
                                        ps[:, half * TQ:(half + 1) * TQ],
                                        k83[t][bp:bp + 32, :,
                                               c * P:(c + 1) * P],
                                        qsl, perf_mode=DR)
                                e = es_pool.tile([P, 2 * TQ], f8, tag="es",
                                                 name="es")
                                with nc.allow_low_precision(reason="fp8 es"):
                                    nc.scalar.activation(
                                        e[:], ps[:], AF.Exp,
                                        scale=1.0 / np.sqrt(HEAD_DIM))
                                es.append(e)
                            es_store[hd] = es
                        if step >= PIPE:
                            hd = step - PIPE
                            ct, ro = hd // 2, (hd % 2) * 64
                            es = es_store.pop(hd)
                            pv = pv_psp.tile([65, TQ], f32, tag="pv",
                                             name="pv_ps")
                            for j in range(8):
                                e3 = es[j].rearrange("p (i t) -> p i t",
                                                     i=2)
                                nc.tensor.matmul(
                                    pv[:], v84[j][:, :, hd, :],
                                    e3[:], perf_mode=DR,
                                    start=(j == 0), stop=(j == 7))
                            # un-normalized head output + its denominator
                            nc.scalar.copy(attn_sb[ct][ro:ro + 64, :],
                                           pv[0:64, :])
                            dtmp = x_pool.tile([1, TQ], bf16,
                                               tag=f"x{hd % 8}",
                                               name=f"dtmp{hd}")
                            with nc.allow_low_precision(reason="bf16 den"):
                                nc.vector.tensor_copy(dtmp[:], pv[64:65, :])
                            nc.sync.dma_start(dden[hd:hd + 1, :], dtmp[:])

                    # batched softmax normalization: one reciprocal for all
                    # 16 heads, then per-tile PE broadcast + multiply
                    recip = attn_pool.tile([16, TQ], bf16, tag="rc",
                                           name="recip")
                    with nc.allow_low_precision(reason="bf16 recip bcast"):
                        nc.vector.reciprocal(recip[:], dden[:])
                    for ct in range(NCC):
                        rb = pv_psp.tile([P, TQ], f32, tag="pv",
                                         name="rb_ps")
                        nc.tensor.matmul(rb[:],
                                         sel16[:, ct * P:(ct + 1) * P],
                                         recip[:])
                        nc.vector.tensor_mul(attn_sb[ct][:], attn_sb[ct][:],
                                             rb[:])

                # xo borrows the v slots (v is dead after the last PV matmul)
                xo = [v_pool.tile([P, TQ], f32, tag=f"v{i}", name=f"xo{i}")
                      for i in range(NCC)]
                for ci in range(NCC):
                    nc.sync.dma_start(xo[ci][:], xo_d[ci * P:(ci + 1) * P, :])

                with tc.tile_pool(name="ops", bufs=4, space="PSUM") as o_ps:
                    for co in range(NCC):
                        ps = o_ps.tile([P, TQ], f32, tag="ps", name="o_ps")
                        for hi in range(NCC):
                            nc.tensor.matmul(ps[:],
                                             ow[hi][:, co * P:(co + 1) * P],
                                             attn_sb[hi][:], start=(hi == 0),
                                             stop=(hi == NCC - 1))
                        nc.vector.scalar_tensor_tensor(x2[co][:], ps[:],
                                                       bcol(OB, co),
                                                       xo[co][:],
                                                       OP.add, OP.add)
                        nc.vector.tensor_copy(xbc[co][:], x2[co][:])

        # ------- LN2 stats over x2 [1024, 512]; xbc is then centered in
        # place so FFN1 is a plain chain: h1 = gelu(s2 * (W1'.xc2) + b1')
        with tc.tile_pool(name="xq2", bufs=2) as xqp, \
             tc.tile_pool(name="ln2ps", bufs=1, space="PSUM") as ln2ps, \
             tc.tile_pool(name="ln2bc", bufs=2, space="PSUM") as ln2bc, \
             tc.tile_pool(name="ln2t", bufs=2) as ln2t:
            st2 = ln2ps.tile([33, TQ], f32, tag="st2", name="st2")
            for ci in range(NCC):
                xq = xqp.tile([P, TQ], bf16, tag="xq", name="xq2")
                nc.vector.tensor_mul(xq[:], xbc[ci][:], xbc[ci][:])
                nc.tensor.matmul(st2[0:1, :], ones_bf[:], xbc[ci][:],
                                 start=(ci == 0), stop=(ci == NCC - 1))
                nc.tensor.matmul(st2[32:33, :], ones_bf[:], xq[:],
                                 start=(ci == 0), stop=(ci == NCC - 1))
            inv_n = 1.0 / D_MODEL
            mu2_sb = ln2t.tile([1, TQ], f32, tag="mu", name="mu2_sb")
            nc.vector.tensor_scalar_mul(mu2_sb[:], st2[0:1, :], inv_n)
            mu2sq = ln2t.tile([1, TQ], f32, tag="musq", name="mu2sq")
            nc.vector.tensor_mul(mu2sq[:], mu2_sb[:], mu2_sb[:])
            vpe = ln2t.tile([1, TQ], f32, tag="vpe", name="vpe2")
            nc.vector.tensor_scalar(vpe[:], st2[32:33, :], inv_n, 1e-5,
                                    OP.mult, OP.add)
            nc.vector.tensor_sub(vpe[:], vpe[:], mu2sq[:])
            rec2 = ln2t.tile([1, TQ], f32, tag="rec2", name="rec2")
            nc.vector.reciprocal_approx_fast(rec2[:], vpe[:])
            s2_bf = ln2t.tile([1, TQ], bf16, tag="sbf", name="s2_bf")
            with nc.allow_low_precision(reason="bf16 LN2 rows"):
                nc.scalar.sqrt(s2_bf[:], rec2[:])
                nc.vector.tensor_scalar_mul(nmu2_row[:], mu2_sb[:], -1.0)
            sb_ps = ln2bc.tile([P, TQ], f32, tag="bc", name="sb2")
            nc.tensor.matmul(sb_ps[:], ones_row[:], s2_bf[:])
            nc.scalar.copy(s2_bb[:], sb_ps[:])
            nm2_ps = ln2bc.tile([P, TQ], f32, tag="bc", name="nm2")
            nc.tensor.matmul(nm2_ps[:], ones_row[:], nmu2_row[:])
            nm2_b = ln2t.tile([P, TQ], bf16, tag="nm2b", name="nm2_b")
            nc.scalar.copy(nm2_b[:], nm2_ps[:])
            for ci in range(NCC):
                nc.vector.tensor_add(xbc[ci][:], xbc[ci][:], nm2_b[:])

        # ---------------- FFN ----------------
        # h1 [4096, 512] lives in the v/xo slots (8 tiles) + a dedicated
        # arena (8 more); w1 streams through the K arena (k dead after
        # the last scores)
        hg = [v_pool.tile([P, 2 * TQ], bf16, tag=f"v{i}", name=f"hg{i}")
              for i in range(8)]
        hg += [h2_pool.tile([P, 2 * TQ], bf16, tag=f"h{i}", name=f"hg{i+8}")
               for i in range(8)]

        def h1sl(fch):
            return hg[fch // 2][:, (fch % 2) * TQ:(fch % 2 + 1) * TQ]

        with tc.tile_pool(name="h1ps", bufs=4, space="PSUM") as h1_ps, \
             tc.tile_pool(name="drt", bufs=4) as drt_pool:
            for qtr in range(4):
                if qtr == 0:
                    w1t = w1q
                else:
                    w1t = []
                    for ci in range(NCC):
                        wt = karena.tile([P, 1024], bf16, tag=f"k{ci}",
                                         name=f"w1t{ci}q{qtr}")
                        nc.sync.dma_start(
                            wt[:], w1_d[ci * P:(ci + 1) * P,
                                        qtr * 1024:(qtr + 1) * 1024])
                        w1t.append(wt)
                for fo in range(8):
                    fch = qtr * 8 + fo
                    ps = h1_ps.tile([P, TQ], f32, tag="ps", name="h1_ps")
                    for ci in range(NCC):
                        nc.tensor.matmul(ps[:],
                                         w1t[ci][:, fo * P:(fo + 1) * P],
                                         xbc[ci][:], start=(ci == 0),
                                         stop=(ci == NCC - 1))
                    drt = drt_pool.tile([P, TQ], bf16, tag="drt", name="drt")
                    nc.vector.tensor_mul(drt[:], ps[:], s2_bb[:])
                    nc.scalar.activation(h1sl(fch), drt[:], AF.Gelu,
                                         bias=bcol(B1, fch))

        with tc.tile_pool(name="outps", bufs=1, space="PSUM") as out_ps, \
             tc.tile_pool(name="outsb", bufs=1) as out_pool:
            ops = [out_ps.tile([P, TQ], f32, tag=f"o{co}", name=f"out_ps{co}")
                   for co in range(NCC)]
            for fch in range(NFC):
                wt = wa_pool.tile([P, D_MODEL], bf16, tag=f"wa{fch % 8}",
                                  name=f"w2t{fch}")
                nc.sync.dma_start(wt[:], w2_d[fch * P:(fch + 1) * P, :])
                for co in range(NCC):
                    nc.tensor.matmul(ops[co][:], wt[:, co * P:(co + 1) * P],
                                     h1sl(fch),
                                     start=(fch == 0), stop=(fch == NFC - 1))
            for co in range(NCC):
                osb = out_pool.tile([P, TQ], f32, tag=f"os{co}",
                                    name=f"osb{co}")
                nc.vector.scalar_tensor_tensor(osb[:], ops[co][:],
                                               bcol(B2, co), x2[co][:],
                                               OP.add, OP.add)
                nc.sync.dma_start(out_d[co * P:(co + 1) * P, :], osb[:])


def _prep_inputs(x, qkv_w, qkv_b, o_w, o_b, ln1_g, ln1_b,
                 ffn_w1, ffn_b1, ffn_w2, ffn_b2, ln2_g, ln2_b):
    import ml_dtypes
    bf = ml_dtypes.bfloat16
    f8 = np.float64

    # fold LN gammas into the following projection weights, LN betas and
    # projection biases into per-output-feature constants (data-independent)
    Wg = qkv_w.astype(f8) * ln1_g.astype(f8)[None, :]
    cvec = qkv_w.astype(f8) @ ln1_b.astype(f8) + qkv_b.astype(f8)
    # permute Q/K output dims into the DoubleRow layout: new dim
    # n = 128*t + 64*hp + 2*dd + i  <-  head (2t+hp), head-dim (32*i + dd)
    perm = np.empty(D_MODEL, np.int64)
    for t in range(8):
        for hp in range(2):
            for dd in range(32):
                for i in range(2):
                    n = t * 128 + hp * 64 + dd * 2 + i
                    perm[n] = (2 * t + hp) * 64 + 32 * i + dd
    WgT = np.ascontiguousarray(Wg.T.astype(np.float32))
    WgT[:, 0:D_MODEL] = WgT[:, perm]
    WgT[:, D_MODEL:2 * D_MODEL] = WgT[:, D_MODEL + perm]
    qkv_wT = WgT.astype(bf)
    cq_perm = cvec[0:D_MODEL][perm].astype(np.float32)
    ob_eff = (o_b.astype(f8) + o_w.astype(f8) @ cvec[2 * D_MODEL:]
              ).astype(np.float32)

    W1g = ffn_w1.astype(f8) * ln2_g.astype(f8)[None, :]
    b1_eff = (ffn_w1.astype(f8) @ ln2_b.astype(f8)
              + ffn_b1.astype(f8)).astype(np.float32)
    w1T = np.ascontiguousarray(W1g.T.astype(np.float32)).astype(bf)

    o_wT = np.ascontiguousarray(o_w.T).astype(bf)
    w2T = np.ascontiguousarray(ffn_w2.T).astype(bf)

    def cols(v, n):
        return np.ascontiguousarray(v.reshape(n, P).T.astype(np.float32))

    biases = np.zeros((P, 64), np.float32)
    biases[:, QB:QB + 8] = cols(cq_perm, 8)
    biases[:, OB:OB + 8] = cols(ob_eff, 8)
    biases[:, B1:B1 + 32] = cols(b1_eff, 32)
    biases[:, B2:B2 + 8] = cols(ffn_b2, 8)

    sel16 = np.zeros((16, NCC * P), np.float32)
    for i in range(16):
        base = (i // 2) * P + (i % 2) * 64
        sel16[i, base:base + 64] = 1.0
    sel16 = sel16.astype(bf)

    in_maps = []
    for c in range(N_CORES):
        b, s = c // GROUPS, c % GROUPS
        xr = np.ascontiguousarray(x[b][s * TQ:(s + 1) * TQ, :].T)
        in_maps.append({
            "x_fm": xr.astype(bf),
            "x_own": xr,
            "qkv_wT": qkv_wT,
            "o_wT": o_wT,
            "w1T": w1T,
            "w2T": w2T,
            "biases": biases,
            "sel16": sel16,
        })
    return in_maps


def kernel(**inputs):
    from concourse.bass_utils import run_bass_kernel_spmd

    if "nc" not in _cache:
        _cache["nc"] = _build()
    nc = _cache["nc"]

    inputs = {k: np.asarray(v, dtype=np.float32) for k, v in inputs.items()}
    in_maps = _prep_inputs(**inputs)

    res = run_bass_kernel_spmd(nc, in_maps, core_ids=list(range(N_CORES)),
                               **_cache.get("run_kwargs", {}))
    _cache["last_results"] = res

    out = np.empty((B, T, D_MODEL), np.float32)
    for c in range(N_CORES):
        b, s = c // GROUPS, c % GROUPS
        out[b, s * TQ:(s + 1) * TQ, :] = res.results[c]["out"].T
    return out
